# revision 1
# baseline (speedup 1.0000x reference)
"""BiLevelRoutingAttention Trainium2 kernel.

Strategy (8 NeuronCores, data-parallel over batch: 2 batches/core, 32 (b,t)
tiles per core):
  - Host: transpose x to feature-major bf16, exact fp32 window-sums of x
    (linearity: region features = (sum_win x) @ W), cast weights to bf16.
  - Device, per (b,t) tile, all layouts feature-major ("T-layout"):
      qT/kT = W^T x^T (bf16 matmuls, fp32 PSUM), V token-major.
      Routing in fp32r from the exact window sums -> sim -> top-4 via max8 ->
      additive window mask, expanded onto scores inside PSUM by a tiny
      matmul (maskW as weights, 0/1 expansion constant as moving operand).
      scoresT += mask, exp on ACT (scale folded), Z via ones-matmuls
      (col-packed), reciprocal + broadcast via SBUF->SBUF DMA, PV col-packed,
      normalize, out-projection, bias, store.
"""

import sys

sys.path.insert(0, "/opt/trn_rl_repo")

import numpy as np
import ml_dtypes

import concourse.bass as bass
import concourse.bacc as bacc
import concourse.mybir as mybir
import concourse.tile as tile
from concourse.bass_utils import run_bass_kernel_spmd

BF16 = mybir.dt.bfloat16
F32 = mybir.dt.float32
F32R = mybir.dt.float32r

NCORES = 8
B, T, S, C = 16, 16, 256, 256
NW, WIN, NH, D, TK = 8, 32, 8, 32, 4
BPC = B // NCORES  # batches per core
SCALE = float(D) ** -0.5
MASKVAL = -1e9

_CACHE = {}


def _build_nc(nt=T):
    nc = bacc.Bacc("TRN2", target_bir_lowering=False, debug=False)

    xt_d = nc.dram_tensor("xt", [BPC, nt, C, S], BF16, kind="ExternalInput")
    xs_d = nc.dram_tensor("xsumt", [BPC, C, nt, NW], F32, kind="ExternalInput")
    wqk_d = nc.dram_tensor("wqk_bf", [C, 2 * C], BF16, kind="ExternalInput")
    wqkf_d = nc.dram_tensor("wqk_f32", [C, 2 * C], F32, kind="ExternalInput")
    wv_d = nc.dram_tensor("wv_bf", [C, C], BF16, kind="ExternalInput")
    wp_d = nc.dram_tensor("wproj_bf", [C, C], BF16, kind="ExternalInput")
    bqk_d = nc.dram_tensor("bqk_cols", [128, 4], F32, kind="ExternalInput")
    bv_d = nc.dram_tensor("bv_row", [1, C], F32, kind="ExternalInput")
    bvbf_d = nc.dram_tensor("bv_bf", [1, C], BF16, kind="ExternalInput")
    bp_d = nc.dram_tensor("bproj_row", [1, C], F32, kind="ExternalInput")
    e8r_d = nc.dram_tensor("e8r", [128, S], BF16, kind="ExternalInput")
    out_d = nc.dram_tensor("out", [BPC, nt, 2, 128, C], F32, kind="ExternalOutput")

    with tile.TileContext(nc) as tc:
        with (
            tc.tile_pool(name="wpool", bufs=1) as wp,
            tc.tile_pool(name="xpool", bufs=4) as xp,
            tc.tile_pool(name="mid", bufs=3) as mp,
            tc.tile_pool(name="exps", bufs=3) as ep,
            tc.tile_pool(name="b1", bufs=4, space="PSUM") as pb1,
            tc.tile_pool(name="sc", bufs=1, space="PSUM") as psc,
            tc.tile_pool(name="dramp", bufs=2, space="DRAM") as dp,
        ):
            # ---- weights / constants (loaded once) ----
            wqk_sb = wp.tile([128, 2, 2 * C], BF16)
            nc.sync.dma_start(out=wqk_sb, in_=wqk_d.ap().rearrange("(cc p) j -> p cc j", p=128))
            wqkf_sb = wp.tile([128, 2, 2 * C], F32)
            nc.sync.dma_start(out=wqkf_sb, in_=wqkf_d.ap().rearrange("(cc p) j -> p cc j", p=128))
            wv_sb = wp.tile([128, 2, C], BF16)
            nc.sync.dma_start(out=wv_sb, in_=wv_d.ap().rearrange("(cc p) j -> p cc j", p=128))
            wp_sb = wp.tile([128, 2, C], BF16)
            nc.sync.dma_start(out=wp_sb, in_=wp_d.ap().rearrange("(cc p) j -> p cc j", p=128))
            bqk_sb = wp.tile([128, 4], F32)
            nc.sync.dma_start(out=bqk_sb, in_=bqk_d.ap())
            # bias rows pre-broadcast to all 128 partitions (DMA supports
            # partition-step-0 source APs; DVE does not)
            bv_sb = wp.tile([128, C], F32)
            nc.sync.dma_start(out=bv_sb, in_=bv_d.ap().to_broadcast([128, C]))
            bp_sb = wp.tile([128, C], F32)
            nc.sync.dma_start(out=bp_sb, in_=bp_d.ap().to_broadcast([128, C]))
            e8r_sb = wp.tile([128, S], BF16)
            nc.sync.dma_start(out=e8r_sb, in_=e8r_d.ap())
            ones_sb = wp.tile([128, 1], BF16)
            nc.vector.memset(ones_sb, 1.0)
            onesr_sb = wp.tile([1, 128], BF16)
            nc.vector.memset(onesr_sb, 1.0)
            bvr_sb = wp.tile([1, C], BF16)
            nc.sync.dma_start(out=bvr_sb, in_=bvbf_d.ap())

            for b in range(BPC):
                xsb_sb = xp.tile([128, 2, nt, NW], F32, tag="xsb")
                nc.sync.dma_start(
                    out=xsb_sb,
                    in_=xs_d[b].rearrange("(cc p) t n -> p cc t n", p=128))
                for t in range(nt):
                    _emit_tile(nc, tc, xp, mp, ep, pb1, psc, dp,
                               xt_d, xsb_sb, out_d, b, t,
                               wqk_sb, wqkf_sb, wv_sb, wp_sb,
                               bqk_sb, bv_sb, bp_sb, e8r_sb, ones_sb,
                               onesr_sb, bvr_sb)

    nc.compile()
    return nc


def _emit_tile(nc, tc, xp, mp, ep, pb1, psc, dp, xt_d, xs_d, out_d, b, t,
               wqk_sb, wqkf_sb, wv_sb, wp_sb, bqk_sb, bv_sb, bp_sb,
               e8r_sb, ones_sb, onesr_sb, bvr_sb):
    import os
    PHASE = int(os.environ.get("KPHASE", "9"))
    AL = mybir.AluOpType

    def _dump(src_t):
        o = mp.tile([128, 2, C], F32, tag="out")
        nc.vector.tensor_copy(out=o, in_=src_t)
        nc.sync.dma_start(out=out_d[b, t].rearrange("s p c -> p s c"), in_=o)

    # ---- load x^T and window-sums ----
    xt_sb = xp.tile([128, 2, S], BF16, tag="xt")
    nc.sync.dma_start(out=xt_sb, in_=xt_d[b, t].rearrange("(cc p) s -> p cc s", p=128))


    # ---- qT / kT (feature-major); one accumulation group per shared bank ----
    qk_sb = mp.tile([128, 4, S], BF16, tag="qk")
    for half in range(2):  # jb pairs {0,1} and {2,3} share a bank each
        qps = pb1.tile([128, 2, S], F32, tag="b1")
        for j in range(2):
            jb = 2 * half + j
            for cc in range(2):
                nc.tensor.matmul(qps[:, j, :],
                                 lhsT=wqk_sb[:, cc, jb * 128:(jb + 1) * 128],
                                 rhs=xt_sb[:, cc, :],
                                 start=(j == 0 and cc == 0),
                                 stop=(j == 1 and cc == 1))
        nc.vector.tensor_tensor(
            out=qk_sb[:, 2 * half:2 * half + 2, :], in0=qps,
            in1=bqk_sb[:, 2 * half:2 * half + 2].unsqueeze(-1)
                .to_broadcast([128, 2, S]),
            op=AL.add)

    # ---- V (token-major), both blocks in one bank, copy on ACT ----
    v_sb = mp.tile([128, 2, C], BF16, tag="v")
    vps = pb1.tile([128, 2, C], F32, tag="b1")
    for sb_ in range(2):
        for cc in range(2):
            nc.tensor.matmul(vps[:, sb_, :],
                             lhsT=xt_sb[:, cc, sb_ * 128:(sb_ + 1) * 128],
                             rhs=wv_sb[:, cc, :],
                             start=(sb_ == 0 and cc == 0), stop=False)
        nc.tensor.matmul(vps[:, sb_, :], lhsT=onesr_sb, rhs=bvr_sb,
                         start=False, stop=(sb_ == 1))
    nc.scalar.activation(out=v_sb, in_=vps,
                         func=mybir.ActivationFunctionType.Copy)

    if PHASE <= 1:
        _dump(v_sb)
        return
    # ---- routing: region features (fp32, exact window sums) + sim ----
    # rg occupies [:, 0:32], sim diag bands [:, 32:40]/[40:48]; the rg group
    # start pre-zeroes the whole bank so unwritten sim partitions read 0
    rs_ps = pb1.tile([128, 2, S], F32, tag="b1")
    for jb in range(4):
        for cc in range(2):
            nc.tensor.matmul(rs_ps[:, 0, jb * 8:(jb + 1) * 8],
                             lhsT=wqkf_sb[:, cc, jb * 128:(jb + 1) * 128],
                             rhs=xs_d[:, cc, t, :],
                             start=(jb == 0 and cc == 0),
                             stop=(jb == 3 and cc == 1))
    rg_sb = mp.tile([128, 4, NW], F32, tag="rg")
    nc.vector.tensor_copy(out=rg_sb,
                          in_=rs_ps[:, 0, 0:32].rearrange("p (a n) -> p a n", n=NW))
    mw_sb = mp.tile([128, 2, NW], BF16, tag="mw")
    mwx_sb = mp.tile([128, 2, S], BF16, tag="mwx")
    nc.vector.memset(rs_ps[:, 0, 32:48], 0.0)
    for jbq in range(2):
        for rg in range(4):
            nc.tensor.matmul(rs_ps[32 * rg:32 * rg + 8, 0,
                                   32 + 8 * jbq:40 + 8 * jbq],
                             lhsT=rg_sb[32 * rg:32 * rg + 32, jbq, :],
                             rhs=rg_sb[32 * rg:32 * rg + 32, 2 + jbq, :],
                             start=False, stop=False,
                             skip_group_check=True,
                             tile_position=(32 * rg, 32 * rg))
    for jbq in range(2):
        mx = mp.tile([128, 8], F32, tag="mx")
        nc.vector.max(out=mx, in_=rs_ps[:, 0, 32 + 8 * jbq:40 + 8 * jbq])
        nc.vector.tensor_scalar(out=mw_sb[:, jbq, :],
                                in0=rs_ps[:, 0, 32 + 8 * jbq:40 + 8 * jbq],
                                scalar1=mx[:, 3:4], scalar2=None, op0=AL.is_ge)
        nc.vector.tensor_scalar(out=mw_sb[:, jbq, :], in0=mw_sb[:, jbq, :],
                                scalar1=1.0, scalar2=-MASKVAL, op0=AL.subtract,
                                op1=AL.mult)
        # window-expand mask on gpsimd (SBUF-only streaming copy)
        nc.gpsimd.tensor_copy(
            out=mwx_sb[:, jbq, :],
            in_=mw_sb[:, jbq, :].unsqueeze(-1).to_broadcast([128, NW, WIN]))

    if PHASE <= 2:
        _dump(mwx_sb)
        return
    # ---- scores^T + mask, exp ----
    # concurrent row-group matmuls must write different PSUM banks; each head
    # rg owns a 512-wide bank holding both kb halves (same row group ->
    # serialized drains)
    expT = ep.tile([128, 2, 4, 2 * S], BF16, tag="expT")
    for jbq in range(2):
        sc_ps = psc.tile([128, 4, 2 * S], F32, tag="sc")
        for kb in range(2):
            for rg in range(4):
                nc.tensor.matmul(
                    sc_ps[:, rg, kb * S:(kb + 1) * S],
                    lhsT=qk_sb[32 * rg:32 * rg + 32, 2 + jbq, kb * 128:(kb + 1) * 128],
                    rhs=qk_sb[32 * rg:32 * rg + 32, jbq, :],
                    start=(kb == 0), stop=False,
                    skip_group_check=True, tile_position=(32 * rg, 0))
                nc.tensor.matmul(
                    sc_ps[:, rg, kb * S:(kb + 1) * S],
                    lhsT=mwx_sb[32 * rg:32 * rg + 8, jbq,
                                kb * 128:(kb + 1) * 128],
                    rhs=e8r_sb[32 * rg:32 * rg + 8, :],
                    start=False, stop=(kb == 1),
                    skip_group_check=True, tile_position=(32 * rg, 0))
        nc.scalar.activation(out=expT[:, jbq, :, :], in_=sc_ps,
                             func=mybir.ActivationFunctionType.Exp,
                             scale=SCALE)

    if PHASE <= 3:
        _dump(expT[:, :, 0, 0:C].rearrange("p a c -> p a c"))
        return
    # ---- Z (col-packed ones-matmuls, both quads in one bank) ----
    zp = pb1.tile([128, 2, S], F32, tag="b1")
    nc.vector.memset(zp, 1.0)  # define non-Z rows for the full-tile recip
    for jbq in range(2):
        for rg in range(4):
            for kb in range(2):
                nc.tensor.matmul(zp[32 * rg:32 * rg + 1, jbq, :],
                                 lhsT=ones_sb,
                                 rhs=expT[:, jbq, rg, kb * S:(kb + 1) * S],
                                 start=(jbq == 0 and kb == 0),
                                 stop=(jbq == 1 and kb == 1),
                                 skip_group_check=True,
                                 tile_position=(0, 32 * rg))
    zrf_sb = mp.tile([128, 2, S], F32, tag="zrf")
    nc.vector.reciprocal(out=zrf_sb, in_=zp)
    # partition-broadcast needs a DRAM source: bounce the 8 recip rows
    # through DRAM, then one broadcast-load expands each row to 32 partitions
    zall_d = dp.tile([4, 2, S], F32, tag="zd")
    nc.scalar.dma_start(
        out=zall_d,
        in_=zrf_sb[:].rearrange("(a c) j q -> a c j q", c=32)[:, 0, :, :])
    rf_sb = mp.tile([128, 2, S], F32, tag="rf")
    for rg in range(4):
        nc.scalar.dma_start(
            out=rf_sb[32 * rg:32 * rg + 32, :, :],
            in_=zall_d[rg].unsqueeze(0).to_broadcast([32, 2, S]))

    if PHASE <= 4:
        _dump(rf_sb)
        return
    # ---- PV (col-packed, both quads in one bank) + normalize ----
    atn_sb = mp.tile([128, 2, S], BF16, tag="atn")
    at = pb1.tile([128, 2, S], F32, tag="b1")
    for jbq in range(2):
        for rg in range(4):
            hh = 4 * jbq + rg
            for kb in range(2):
                nc.tensor.matmul(at[32 * rg:32 * rg + 32, jbq, :],
                                 lhsT=v_sb[:, kb, 32 * hh:32 * hh + 32],
                                 rhs=expT[:, jbq, rg, kb * S:(kb + 1) * S],
                                 start=(jbq == 0 and kb == 0),
                                 stop=(jbq == 1 and kb == 1),
                                 skip_group_check=True,
                                 tile_position=(0, 32 * rg))
    nc.vector.tensor_tensor(out=atn_sb, in0=at, in1=rf_sb, op=AL.mult)

    # ---- out projection (both s-blocks in one bank) ----
    out_sb = mp.tile([128, 2, C], F32, tag="out")
    po = pb1.tile([128, 2, C], F32, tag="b1")
    for sb_ in range(2):
        for cc in range(2):
            nc.tensor.matmul(po[:, sb_, :],
                             lhsT=atn_sb[:, cc, sb_ * 128:(sb_ + 1) * 128],
                             rhs=wp_sb[:, cc, :],
                             start=(sb_ == 0 and cc == 0),
                             stop=(sb_ == 1 and cc == 1))
    nc.vector.tensor_tensor(out=out_sb, in0=po,
                            in1=bp_sb[:].unsqueeze(1).to_broadcast([128, 2, C]),
                            op=AL.add)
    nc.sync.dma_start(out=out_d[b, t].rearrange("s p c -> p s c"),
                      in_=out_sb)


def _host_prep(x, w_qkv, b_qkv, w_proj, b_proj):
    bf16 = ml_dtypes.bfloat16
    x4 = x.reshape(B, T, S, C)
    xt = np.ascontiguousarray(x4.transpose(0, 1, 3, 2)).astype(bf16)
    xsum = x4.reshape(B, T, NW, WIN, C).sum(3, dtype=np.float64).astype(np.float32)
    xsumt = np.ascontiguousarray(xsum.transpose(0, 3, 1, 2))  # [B, C, T, NW]

    shared = {
        "wqk_bf": np.ascontiguousarray(w_qkv[:, :2 * C]).astype(bf16),
        "wqk_f32": np.ascontiguousarray(w_qkv[:, :2 * C]).astype(np.float32),
        "wv_bf": np.ascontiguousarray(w_qkv[:, 2 * C:]).astype(bf16),
        "wproj_bf": w_proj.astype(bf16),
        "bqk_cols": np.ascontiguousarray(
            b_qkv[:2 * C].reshape(4, 128).T).astype(np.float32),
        "bv_row": b_qkv[2 * C:].reshape(1, C).astype(np.float32),
        "bv_bf": b_qkv[2 * C:].reshape(1, C).astype(bf16),
        "bproj_row": b_proj.reshape(1, C).astype(np.float32),
        "e8r": _make_e8r(),
    }
    in_maps = []
    for core in range(NCORES):
        b0 = core * BPC
        m = dict(shared)
        m["xt"] = np.ascontiguousarray(xt[b0:b0 + BPC])
        m["xsumt"] = np.ascontiguousarray(xsumt[b0:b0 + BPC])
        in_maps.append(m)
    return in_maps


def _make_e8r():
    e = np.zeros((128, S), ml_dtypes.bfloat16)
    q = np.arange(S) // WIN  # query window of column q
    for rg in range(4):
        for n in range(NW):
            e[32 * rg + n, q == n] = 1.0
    return e


def kernel(x, w_qkv, b_qkv, w_proj, b_proj, **_unused_scalars):
    x = np.asarray(x, dtype=np.float32)
    w_qkv = np.asarray(w_qkv, dtype=np.float32)
    b_qkv = np.asarray(b_qkv, dtype=np.float32)
    w_proj = np.asarray(w_proj, dtype=np.float32)
    b_proj = np.asarray(b_proj, dtype=np.float32)

    if "nc" not in _CACHE:
        _CACHE["nc"] = _build_nc()
    nc = _CACHE["nc"]

    in_maps = _host_prep(x, w_qkv, b_qkv, w_proj, b_proj)
    res = run_bass_kernel_spmd(nc, in_maps, core_ids=list(range(NCORES)))

    out = np.empty((B, T, 2, 128, C), np.float32)
    for core in range(NCORES):
        out[core * BPC:(core + 1) * BPC] = res.results[core]["out"]
    # [B, T, sb, p, C] -> [B, T*S, C]
    return out.reshape(B, T * S, C)



# revision 3
# speedup vs baseline: 2.1680x; 2.1680x over previous
"""BiLevelRoutingAttention Trainium2 kernel (v2).

Strategy (8 NeuronCores, data-parallel over batch: 2 batches/core, 32 (b,t)
tiles per core):
  - Host: transpose x to feature-major bf16; ROUTING ON HOST (fp64 window
    sums -> region features -> sim -> top-4 -> additive window mask), mask
    uploaded pre-expanded as a [16, 128] bf16 matmul lhsT per (tile, jbq)
    that a single N=512 matmul (vs static one-hot e16r rhs) expands onto
    the scores inside PSUM.
  - Device, per (b,t) tile, all layouts feature-major ("T-layout"):
      qT/kT = W^T x^T (bf16 matmuls, fp32 PSUM), V token-major.
      scoresT + mask accumulated in PSUM, exp on ACT (scale folded),
      Z via ones[128,32]-matmuls -> Z broadcast in PSUM (no DRAM bounce),
      reciprocal_approx_fast (DVE), PV col-packed, normalize, out
      projection, store fp32.
"""

import sys

sys.path.insert(0, "/opt/trn_rl_repo")

import numpy as np
import ml_dtypes

import concourse.bass as bass
import concourse.bacc as bacc
import concourse.mybir as mybir
import concourse.tile as tile
from concourse.bass_utils import run_bass_kernel_spmd

BF16 = mybir.dt.bfloat16
F32 = mybir.dt.float32

NCORES = 8
B, T, S, C = 16, 16, 256, 256
NW, WIN, NH, D, TK = 8, 32, 8, 32, 4
BPC = B // NCORES  # batches per core
SCALE = float(D) ** -0.5
MASKVAL = -1e9

_CACHE = {}


def _build_nc(has_bqk, has_bf, nt=T):
    nc = bacc.Bacc("TRN2", target_bir_lowering=False, debug=False)

    xt_d = nc.dram_tensor("xt", [BPC, nt, C, S], BF16, kind="ExternalInput")
    mw_d = nc.dram_tensor("mw16", [BPC, nt, 128, 2, 128], BF16,
                          kind="ExternalInput")
    wqk_d = nc.dram_tensor("wqk_bf", [C, 2 * C], BF16, kind="ExternalInput")
    wv_d = nc.dram_tensor("wv_bf", [C, C], BF16, kind="ExternalInput")
    wp_d = nc.dram_tensor("wproj_bf", [C, C], BF16, kind="ExternalInput")
    e16_d = nc.dram_tensor("e16r", [128, 2 * S], BF16, kind="ExternalInput")
    bqk_d = nc.dram_tensor("bqk_cols", [128, 4], F32, kind="ExternalInput")
    bf_d = nc.dram_tensor("bfinal_row", [1, C], BF16, kind="ExternalInput")
    out_d = nc.dram_tensor("out", [BPC, nt, 2, 128, C], F32,
                           kind="ExternalOutput")

    with tile.TileContext(nc) as tc:
        with (
            tc.tile_pool(name="wpool", bufs=1) as wp,
            tc.tile_pool(name="xpool", bufs=4) as xp,
            tc.tile_pool(name="mid", bufs=3) as mp,
            tc.tile_pool(name="exps", bufs=3) as ep,
            tc.tile_pool(name="b1", bufs=4, space="PSUM") as pb1,
            tc.tile_pool(name="sc", bufs=1, space="PSUM") as psc,
        ):
            # ---- weights / constants (loaded once) ----
            wqk_sb = wp.tile([128, 2, 2 * C], BF16)
            nc.sync.dma_start(out=wqk_sb,
                              in_=wqk_d.ap().rearrange("(cc p) j -> p cc j", p=128))
            wv_sb = wp.tile([128, 2, C], BF16)
            nc.sync.dma_start(out=wv_sb,
                              in_=wv_d.ap().rearrange("(cc p) j -> p cc j", p=128))
            wp_sb = wp.tile([128, 2, C], BF16)
            nc.sync.dma_start(out=wp_sb,
                              in_=wp_d.ap().rearrange("(cc p) j -> p cc j", p=128))
            e16_sb = wp.tile([128, 2 * S], BF16)
            nc.sync.dma_start(out=e16_sb, in_=e16_d.ap())
            ones32_sb = wp.tile([128, 32], BF16)
            nc.vector.memset(ones32_sb, 1.0)
            bqk_sb = None
            if has_bqk:
                bqk_sb = wp.tile([128, 4], F32)
                nc.sync.dma_start(out=bqk_sb, in_=bqk_d.ap())
            bf_sb = onesr_sb = None
            if has_bf:
                bf_sb = wp.tile([1, C], BF16)
                nc.sync.dma_start(out=bf_sb, in_=bf_d.ap())
                onesr_sb = wp.tile([1, 128], BF16)
                nc.vector.memset(onesr_sb, 1.0)

            for b in range(BPC):
                for t in range(nt):
                    _emit_tile(nc, xp, mp, ep, pb1, psc,
                               xt_d, mw_d, out_d, b, t,
                               wqk_sb, wv_sb, wp_sb, e16_sb, ones32_sb,
                               bqk_sb, bf_sb, onesr_sb)

    nc.compile()
    return nc


def _emit_tile(nc, xp, mp, ep, pb1, psc, xt_d, mw_d, out_d, b, t,
               wqk_sb, wv_sb, wp_sb, e16_sb, ones32_sb,
               bqk_sb, bf_sb, onesr_sb):
    AL = mybir.AluOpType

    # ---- load x^T and mask lhsT ----
    xt_sb = xp.tile([128, 2, S], BF16, tag="xt")
    nc.sync.dma_start(out=xt_sb,
                      in_=xt_d[b, t].rearrange("(cc p) s -> p cc s", p=128))
    mw_sb = xp.tile([128, 2, 128], BF16, tag="mw")
    nc.sync.dma_start(out=mw_sb, in_=mw_d[b, t])

    # ---- qT / kT (feature-major); one accumulation group per shared bank ----
    qk_sb = mp.tile([128, 4, S], BF16, tag="qk")
    for half in range(2):  # jb pairs {0,1} and {2,3} share a bank each
        qps = pb1.tile([128, 2, S], F32, tag="b1")
        for j in range(2):
            jb = 2 * half + j
            for cc in range(2):
                nc.tensor.matmul(qps[:, j, :],
                                 lhsT=wqk_sb[:, cc, jb * 128:(jb + 1) * 128],
                                 rhs=xt_sb[:, cc, :],
                                 start=(j == 0 and cc == 0),
                                 stop=(j == 1 and cc == 1))
        if bqk_sb is not None:
            nc.vector.tensor_tensor(
                out=qk_sb[:, 2 * half:2 * half + 2, :], in0=qps,
                in1=bqk_sb[:, 2 * half:2 * half + 2].unsqueeze(-1)
                    .to_broadcast([128, 2, S]),
                op=AL.add)
        else:
            nc.vector.tensor_copy(out=qk_sb[:, 2 * half:2 * half + 2, :],
                                  in_=qps)

    # ---- V (token-major), both blocks in one bank ----
    v_sb = mp.tile([128, 2, C], BF16, tag="v")
    vps = pb1.tile([128, 2, C], F32, tag="b1")
    for sb_ in range(2):
        for cc in range(2):
            nc.tensor.matmul(vps[:, sb_, :],
                             lhsT=xt_sb[:, cc, sb_ * 128:(sb_ + 1) * 128],
                             rhs=wv_sb[:, cc, :],
                             start=(sb_ == 0 and cc == 0),
                             stop=(sb_ == 1 and cc == 1))
    nc.vector.tensor_copy(out=v_sb, in_=vps)

    # ---- scores^T + mask in PSUM, exp on ACT ----
    # per jbq: 4 head row-groups run concurrently; per (rg, kb) the score
    # matmul accumulates, then one N=512 matmul (mask lhsT x one-hot e16r)
    # adds the additive window mask for both kb halves at once
    expT = ep.tile([128, 2, 4, 2 * S], BF16, tag="expT")
    for jbq in range(2):
        sc_ps = psc.tile([128, 4, 2 * S], F32, tag="sc")
        for rg in range(4):
            for kb in range(2):
                nc.tensor.matmul(
                    sc_ps[:, rg, kb * S:(kb + 1) * S],
                    lhsT=qk_sb[32 * rg:32 * rg + 32, 2 + jbq,
                               kb * 128:(kb + 1) * 128],
                    rhs=qk_sb[32 * rg:32 * rg + 32, jbq, :],
                    start=(kb == 0), stop=False,
                    skip_group_check=True, tile_position=(32 * rg, 0))
            nc.tensor.matmul(
                sc_ps[:, rg, :],
                lhsT=mw_sb[32 * rg:32 * rg + 16, jbq, :],
                rhs=e16_sb[32 * rg:32 * rg + 16, :],
                start=False, stop=True,
                skip_group_check=True, tile_position=(32 * rg, 0))
        nc.scalar.activation(out=expT[:, jbq, :, :], in_=sc_ps,
                             func=mybir.ActivationFunctionType.Exp,
                             scale=SCALE)

    # ---- Z broadcast in PSUM (ones[128,32] lhsT -> 32 replicated rows) ----
    zp = pb1.tile([128, 2, S], F32, tag="b1")
    for jbq in range(2):
        for rg in range(4):
            for kb in range(2):
                nc.tensor.matmul(zp[32 * rg:32 * rg + 32, jbq, :],
                                 lhsT=ones32_sb,
                                 rhs=expT[:, jbq, rg, kb * S:(kb + 1) * S],
                                 start=(jbq == 0 and kb == 0),
                                 stop=(jbq == 1 and kb == 1),
                                 skip_group_check=True,
                                 tile_position=(0, 32 * rg))
    rf_sb = mp.tile([128, 2, S], F32, tag="rf")
    nc.vector.reciprocal_approx_fast(out=rf_sb, in_=zp)

    # ---- PV (col-packed, both quads in one bank) + normalize ----
    atn_sb = mp.tile([128, 2, S], BF16, tag="atn")
    at = pb1.tile([128, 2, S], F32, tag="b1")
    for jbq in range(2):
        for rg in range(4):
            hh = 4 * jbq + rg
            for kb in range(2):
                nc.tensor.matmul(at[32 * rg:32 * rg + 32, jbq, :],
                                 lhsT=v_sb[:, kb, 32 * hh:32 * hh + 32],
                                 rhs=expT[:, jbq, rg, kb * S:(kb + 1) * S],
                                 start=(jbq == 0 and kb == 0),
                                 stop=(jbq == 1 and kb == 1),
                                 skip_group_check=True,
                                 tile_position=(0, 32 * rg))
    nc.vector.tensor_tensor(out=atn_sb, in0=at, in1=rf_sb, op=AL.mult)

    # ---- out projection (both s-blocks in one bank) ----
    out_sb = mp.tile([128, 2, C], F32, tag="out")
    po = pb1.tile([128, 2, C], F32, tag="b1")
    for sb_ in range(2):
        for cc in range(2):
            nc.tensor.matmul(po[:, sb_, :],
                             lhsT=atn_sb[:, cc, sb_ * 128:(sb_ + 1) * 128],
                             rhs=wp_sb[:, cc, :],
                             start=(sb_ == 0 and cc == 0),
                             stop=(bf_sb is None and sb_ == 1 and cc == 1))
        if bf_sb is not None:
            nc.tensor.matmul(po[:, sb_, :], lhsT=onesr_sb, rhs=bf_sb,
                             start=False, stop=(sb_ == 1))
    nc.vector.tensor_copy(out=out_sb, in_=po)
    nc.sync.dma_start(out=out_d[b, t].rearrange("s p c -> p s c"),
                      in_=out_sb)


def _host_routing(x4, w_qkv, b_qkv):
    """Top-4 window routing in fp64 on host -> additive mask lhsT layout
    [B, T, 128, 2, 128] bf16 (rows 32*rg + 8*kb + w = mask of q-window w,
    head 4*jbq+rg, key chunk kb)."""
    xsum = x4.reshape(B, T, NW, WIN, C).sum(3, dtype=np.float64)  # [B,T,NW,C]
    wq = w_qkv[:, :C].astype(np.float64)
    wk = w_qkv[:, C:2 * C].astype(np.float64)
    q_reg = xsum @ wq + WIN * b_qkv[:C].astype(np.float64)
    k_reg = xsum @ wk + WIN * b_qkv[C:2 * C].astype(np.float64)
    # [B,T,NW,h,d]
    q_reg = q_reg.reshape(B, T, NW, NH, D)
    k_reg = k_reg.reshape(B, T, NW, NH, D)
    sim = np.einsum('btnhd,btmhd->bthnm', q_reg, k_reg)  # [B,T,h,NW,NW]
    # top-4 per (b,t,h,qwin): additive mask over kwin
    thr = -np.partition(-sim, TK - 1, axis=-1)[..., TK - 1:TK]
    am = np.where(sim >= thr, 0.0, MASKVAL).astype(np.float32)
    # guard: exact ties could select >4; break by index like lax.top_k
    nsel = (am == 0.0).sum(-1)
    if np.any(nsel != TK):  # pragma: no cover - ties are measure-zero
        idx = np.argsort(-sim, axis=-1, kind='stable')[..., :TK]
        am = np.full(sim.shape, MASKVAL, np.float32)
        np.put_along_axis(am, idx, 0.0, axis=-1)
    # expand: [B,T,h,qw,kwin] -> [B,T,h,qw,256key]
    amx = np.repeat(am, WIN, axis=-1)  # [B,T,h,qw,256]
    mw16 = np.empty((B, T, 128, 2, 128), np.float32)
    for jbq in range(2):
        for rg in range(4):
            h = 4 * jbq + rg
            for kb in range(2):
                # rows 32rg+8kb+w , cols 0..128 keys of chunk kb
                mw16[:, :, 32 * rg + 8 * kb:32 * rg + 8 * kb + 8, jbq, :] = \
                    amx[:, :, h, :, kb * 128:(kb + 1) * 128]
            mw16[:, :, 32 * rg + 16:32 * rg + 32, jbq, :] = 0.0
    return mw16.astype(ml_dtypes.bfloat16)


def _make_e16r():
    e = np.zeros((128, 2 * S), ml_dtypes.bfloat16)
    q = np.arange(S) // WIN  # query window of column q
    for rg in range(4):
        for kb in range(2):
            for w in range(NW):
                e[32 * rg + 8 * kb + w, kb * S + np.arange(S)[q == w]] = 1.0
    return e


def _host_prep(x, w_qkv, b_qkv, w_proj, b_proj):
    bf16 = ml_dtypes.bfloat16
    x4 = x.reshape(B, T, S, C)
    xt = np.ascontiguousarray(x4.transpose(0, 1, 3, 2)).astype(bf16)
    mw16 = _host_routing(x4, w_qkv, b_qkv)

    bfinal = b_qkv[2 * C:] @ w_proj + b_proj
    shared = {
        "wqk_bf": np.ascontiguousarray(w_qkv[:, :2 * C]).astype(bf16),
        "wv_bf": np.ascontiguousarray(w_qkv[:, 2 * C:]).astype(bf16),
        "wproj_bf": w_proj.astype(bf16),
        "e16r": _make_e16r(),
        "bqk_cols": np.ascontiguousarray(
            b_qkv[:2 * C].reshape(4, 128).T).astype(np.float32),
        "bfinal_row": bfinal.reshape(1, C).astype(bf16),
    }
    in_maps = []
    for core in range(NCORES):
        b0 = core * BPC
        m = dict(shared)
        m["xt"] = np.ascontiguousarray(xt[b0:b0 + BPC])
        m["mw16"] = np.ascontiguousarray(mw16[b0:b0 + BPC])
        in_maps.append(m)
    return in_maps


def kernel(x, w_qkv, b_qkv, w_proj, b_proj, **_unused_scalars):
    x = np.asarray(x, dtype=np.float32)
    w_qkv = np.asarray(w_qkv, dtype=np.float32)
    b_qkv = np.asarray(b_qkv, dtype=np.float32)
    w_proj = np.asarray(w_proj, dtype=np.float32)
    b_proj = np.asarray(b_proj, dtype=np.float32)

    has_bqk = bool(np.any(b_qkv[:2 * C]))
    bfinal = b_qkv[2 * C:] @ w_proj + b_proj
    has_bf = bool(np.any(bfinal))
    key = ("nc", has_bqk, has_bf)
    if key not in _CACHE:
        _CACHE[key] = _build_nc(has_bqk, has_bf)
        _CACHE["nc"] = _CACHE[key]
    nc = _CACHE[key]

    in_maps = _host_prep(x, w_qkv, b_qkv, w_proj, b_proj)
    res = run_bass_kernel_spmd(nc, in_maps, core_ids=list(range(NCORES)))

    out = np.empty((B, T, 2, 128, C), np.float32)
    for core in range(NCORES):
        out[core * BPC:(core + 1) * BPC] = res.results[core]["out"]
    # [B, T, sb, p, C] -> [B, T*S, C]
    return out.reshape(B, T * S, C)


# revision 13
# speedup vs baseline: 2.5289x; 1.1664x over previous
"""BiLevelRoutingAttention Trainium2 kernel (v2).

Strategy (8 NeuronCores, data-parallel over batch: 2 batches/core, 32 (b,t)
tiles per core):
  - Host: transpose x to feature-major bf16; ROUTING ON HOST (fp64 window
    sums -> region features -> sim -> top-4 -> additive window mask), mask
    uploaded pre-expanded as a [16, 128] bf16 matmul lhsT per (tile, jbq)
    that a single N=512 matmul (vs static one-hot e16r rhs) expands onto
    the scores inside PSUM.
  - Device, per (b,t) tile, all layouts feature-major ("T-layout"):
      qT/kT = W^T x^T (bf16 matmuls, fp32 PSUM), V token-major.
      scoresT + mask accumulated in PSUM, exp on ACT (scale folded),
      Z via ones[128,32]-matmuls -> Z broadcast in PSUM (no DRAM bounce),
      reciprocal_approx_fast (DVE), PV col-packed, normalize, out
      projection, store fp32.
"""

import sys

sys.path.insert(0, "/opt/trn_rl_repo")

import numpy as np
import ml_dtypes

import concourse.bass as bass
import concourse.bacc as bacc
import concourse.mybir as mybir
import concourse.tile as tile
from concourse.bass_utils import run_bass_kernel_spmd

BF16 = mybir.dt.bfloat16
F32 = mybir.dt.float32

NCORES = 8
B, T, S, C = 16, 16, 256, 256
NW, WIN, NH, D, TK = 8, 32, 8, 32, 4
BPC = B // NCORES  # batches per core
SCALE = float(D) ** -0.5
MASKVAL = -1e9

_CACHE = {}


def _build_nc(has_bqk, has_bf, nt=T):
    nc = bacc.Bacc("TRN2", target_bir_lowering=False, debug=False)

    xt_d = nc.dram_tensor("xt", [BPC, nt, C, S], BF16, kind="ExternalInput")
    mw_d = nc.dram_tensor("mw16", [BPC, nt, 128, 2, 2, 128], BF16,
                          kind="ExternalInput")
    wqk_d = nc.dram_tensor("wqk_bf", [C, 2 * C], BF16, kind="ExternalInput")
    wv_d = nc.dram_tensor("wv_bf", [C, C], BF16, kind="ExternalInput")
    wp_d = nc.dram_tensor("wproj_bf", [C, C], BF16, kind="ExternalInput")
    e16_d = nc.dram_tensor("e16r", [128, S], BF16, kind="ExternalInput")
    bqk_d = nc.dram_tensor("bqk_cols", [128, 4], F32, kind="ExternalInput")
    bf_d = nc.dram_tensor("bfinal_row", [1, C], BF16, kind="ExternalInput")
    out_d = nc.dram_tensor("out", [BPC, nt, 2, 128, C], F32,
                           kind="ExternalOutput")

    with tile.TileContext(nc) as tc:
        with (
            tc.tile_pool(name="wpool", bufs=1) as wp,
            tc.tile_pool(name="xpool", bufs=4) as xp,
            tc.tile_pool(name="mid", bufs=3) as mp,
            tc.tile_pool(name="exps", bufs=3) as ep,
            tc.tile_pool(name="pqk", bufs=2, space="PSUM") as pqk,
            tc.tile_pool(name="pva", bufs=1, space="PSUM") as pva,
            tc.tile_pool(name="pzo", bufs=1, space="PSUM") as pzo,
            tc.tile_pool(name="sc", bufs=1, space="PSUM") as psc,
        ):
            # ---- weights / constants (loaded once) ----
            wqk_sb = wp.tile([128, 2, 2 * C], BF16)
            nc.sync.dma_start(out=wqk_sb,
                              in_=wqk_d.ap().rearrange("(cc p) j -> p cc j", p=128))
            wv_sb = wp.tile([128, 2, C], BF16)
            nc.sync.dma_start(out=wv_sb,
                              in_=wv_d.ap().rearrange("(cc p) j -> p cc j", p=128))
            wp_sb = wp.tile([128, 2, C], BF16)
            nc.sync.dma_start(out=wp_sb,
                              in_=wp_d.ap().rearrange("(cc p) j -> p cc j", p=128))
            e16_sb = wp.tile([128, S], BF16)
            nc.sync.dma_start(out=e16_sb, in_=e16_d.ap())
            ones32_sb = wp.tile([128, 32], BF16)
            nc.vector.memset(ones32_sb, 1.0)
            bqk_sb = None
            if has_bqk:
                bqk_sb = wp.tile([128, 4], F32)
                nc.sync.dma_start(out=bqk_sb, in_=bqk_d.ap())
            bf_sb = onesr_sb = None
            if has_bf:
                bf_sb = wp.tile([1, C], BF16)
                nc.sync.dma_start(out=bf_sb, in_=bf_d.ap())
                onesr_sb = wp.tile([1, 128], BF16)
                nc.vector.memset(onesr_sb, 1.0)

            for b in range(BPC):
                for t in range(nt):
                    _emit_tile(nc, xp, mp, ep, pqk, pva, pzo, psc,
                               xt_d, mw_d, out_d, b, t,
                               wqk_sb, wv_sb, wp_sb, e16_sb, ones32_sb,
                               bqk_sb, bf_sb, onesr_sb)

    nc.compile()
    return nc


def _emit_tile(nc, xp, mp, ep, pqk, pva, pzo, psc, xt_d, mw_d, out_d, b, t,
               wqk_sb, wv_sb, wp_sb, e16_sb, ones32_sb,
               bqk_sb, bf_sb, onesr_sb):
    AL = mybir.AluOpType

    # ---- load x^T and mask lhsT ----
    xt_sb = xp.tile([128, 2, S], BF16, tag="xt")
    nc.sync.dma_start(out=xt_sb,
                      in_=xt_d[b, t].rearrange("(cc p) s -> p cc s", p=128))
    mw_sb = xp.tile([128, 2, 2, 128], BF16, tag="mw")
    nc.sync.dma_start(out=mw_sb[:].rearrange("p a b k -> p (a b k)"),
                      in_=mw_d[b, t].rearrange("p a b k -> p (a b k)"))

    # ---- qT / kT (feature-major); one accumulation group per shared bank ----
    qk_sb = mp.tile([128, 4, S], BF16, tag="qk")
    for half in range(2):  # jb pairs {0,1} and {2,3} share a bank each
        qps = pqk.tile([128, 2, S], F32, tag="qkp")
        for j in range(2):
            jb = 2 * half + j
            for cc in range(2):
                nc.tensor.matmul(qps[:, j, :],
                                 lhsT=wqk_sb[:, cc, jb * 128:(jb + 1) * 128],
                                 rhs=xt_sb[:, cc, :],
                                 start=(j == 0 and cc == 0),
                                 stop=(j == 1 and cc == 1))
        if bqk_sb is not None:
            nc.vector.tensor_tensor(
                out=qk_sb[:, 2 * half:2 * half + 2, :], in0=qps,
                in1=bqk_sb[:, 2 * half:2 * half + 2].unsqueeze(-1)
                    .to_broadcast([128, 2, S]),
                op=AL.add)
        else:
            nc.vector.tensor_copy(out=qk_sb[:, 2 * half:2 * half + 2, :],
                                  in_=qps)

    # ---- V (token-major), both blocks in one bank ----
    v_sb = mp.tile([128, 2, C], BF16, tag="v")
    vps = pva.tile([128, 2, C], F32, tag="va")
    for sb_ in range(2):
        for cc in range(2):
            nc.tensor.matmul(vps[:, sb_, :],
                             lhsT=xt_sb[:, cc, sb_ * 128:(sb_ + 1) * 128],
                             rhs=wv_sb[:, cc, :],
                             start=(sb_ == 0 and cc == 0),
                             stop=(sb_ == 1 and cc == 1))
    nc.vector.tensor_copy(out=v_sb, in_=vps)

    # ---- scores^T + mask in PSUM, exp on ACT ----
    # scores split at (jbq, kb) granularity into 2-bank PSUM tiles
    # (bufs=2) so the ACT exp stream never waits on score matmuls: while
    # exp reads one buffer, the PE fills the other. Per (jbq, kb, rg):
    # one score matmul + one mask-expansion matmul (mask lhsT x one-hot
    # e16r rows) accumulate in PSUM; 4 head row-groups run concurrently.
    expT = ep.tile([128, 2, 4, 2 * S], BF16, tag="expT")
    for jbq in range(2):
        sc_ps = psc.tile([128, 4, 2 * S], F32, tag="sc")
        for rg in range(4):
            for kb in range(2):
                nc.tensor.matmul(
                    sc_ps[:, rg, kb * S:(kb + 1) * S],
                    lhsT=qk_sb[32 * rg:32 * rg + 32, 2 + jbq,
                               kb * 128:(kb + 1) * 128],
                    rhs=qk_sb[32 * rg:32 * rg + 32, jbq, :],
                    start=(kb == 0), stop=False,
                    skip_group_check=True, tile_position=(32 * rg, 0))
                nc.tensor.matmul(
                    sc_ps[:, rg, kb * S:(kb + 1) * S],
                    lhsT=mw_sb[32 * rg:32 * rg + 8, jbq, kb, :],
                    rhs=e16_sb[32 * rg:32 * rg + 8, :],
                    start=False, stop=(kb == 1),
                    skip_group_check=True, tile_position=(32 * rg, 0))
        nc.scalar.activation(out=expT[:, jbq, :, :], in_=sc_ps,
                             func=mybir.ActivationFunctionType.Exp,
                             scale=SCALE)

    # ---- Z broadcast in PSUM (ones[128,32] lhsT -> 32 replicated rows) ----
    zp = pzo.tile([128, 2, S], F32, tag="zo")
    for jbq in range(2):
        for rg in range(4):
            for kb in range(2):
                nc.tensor.matmul(zp[32 * rg:32 * rg + 32, jbq, :],
                                 lhsT=ones32_sb,
                                 rhs=expT[:, jbq, rg, kb * S:(kb + 1) * S],
                                 start=(jbq == 0 and kb == 0),
                                 stop=(jbq == 1 and kb == 1),
                                 skip_group_check=True,
                                 tile_position=(0, 32 * rg))
    rf_sb = mp.tile([128, 2, S], F32, tag="rf")
    nc.vector.reciprocal_approx_fast(out=rf_sb, in_=zp)

    # ---- PV (col-packed, both quads in one bank) + normalize ----
    atn_sb = mp.tile([128, 2, S], BF16, tag="atn")
    at = pva.tile([128, 2, S], F32, tag="va")
    for jbq in range(2):
        for rg in range(4):
            hh = 4 * jbq + rg
            for kb in range(2):
                nc.tensor.matmul(at[32 * rg:32 * rg + 32, jbq, :],
                                 lhsT=v_sb[:, kb, 32 * hh:32 * hh + 32],
                                 rhs=expT[:, jbq, rg, kb * S:(kb + 1) * S],
                                 start=(jbq == 0 and kb == 0),
                                 stop=(jbq == 1 and kb == 1),
                                 skip_group_check=True,
                                 tile_position=(0, 32 * rg))
    nc.vector.tensor_tensor(out=atn_sb, in0=at, in1=rf_sb, op=AL.mult)

    # ---- out projection (both s-blocks in one bank) ----
    out_sb = mp.tile([128, 2, C], F32, tag="out")
    po = pzo.tile([128, 2, C], F32, tag="zo")
    for sb_ in range(2):
        for cc in range(2):
            nc.tensor.matmul(po[:, sb_, :],
                             lhsT=atn_sb[:, cc, sb_ * 128:(sb_ + 1) * 128],
                             rhs=wp_sb[:, cc, :],
                             start=(sb_ == 0 and cc == 0),
                             stop=(bf_sb is None and sb_ == 1 and cc == 1))
        if bf_sb is not None:
            nc.tensor.matmul(po[:, sb_, :], lhsT=onesr_sb, rhs=bf_sb,
                             start=False, stop=(sb_ == 1))
    nc.vector.tensor_copy(out=out_sb, in_=po)
    nc.sync.dma_start(out=out_d[b, t].rearrange("s p c -> p s c"),
                      in_=out_sb)


def _host_routing(x4, w_qkv, b_qkv):
    """Top-4 window routing in fp64 on host -> additive mask lhsT layout
    [B, T, 128, 2, 128] bf16 (rows 32*rg + 8*kb + w = mask of q-window w,
    head 4*jbq+rg, key chunk kb)."""
    xsum = x4.reshape(B, T, NW, WIN, C).sum(3, dtype=np.float64)  # [B,T,NW,C]
    wq = w_qkv[:, :C].astype(np.float64)
    wk = w_qkv[:, C:2 * C].astype(np.float64)
    q_reg = xsum @ wq + WIN * b_qkv[:C].astype(np.float64)
    k_reg = xsum @ wk + WIN * b_qkv[C:2 * C].astype(np.float64)
    # [B,T,NW,h,d]
    q_reg = q_reg.reshape(B, T, NW, NH, D)
    k_reg = k_reg.reshape(B, T, NW, NH, D)
    sim = np.einsum('btnhd,btmhd->bthnm', q_reg, k_reg)  # [B,T,h,NW,NW]
    # top-4 per (b,t,h,qwin): additive mask over kwin
    thr = -np.partition(-sim, TK - 1, axis=-1)[..., TK - 1:TK]
    am = np.where(sim >= thr, 0.0, MASKVAL).astype(np.float32)
    # guard: exact ties could select >4; break by index like lax.top_k
    nsel = (am == 0.0).sum(-1)
    if np.any(nsel != TK):  # pragma: no cover - ties are measure-zero
        idx = np.argsort(-sim, axis=-1, kind='stable')[..., :TK]
        am = np.full(sim.shape, MASKVAL, np.float32)
        np.put_along_axis(am, idx, 0.0, axis=-1)
    # expand: [B,T,h,qw,kwin] -> [B,T,h,qw,256key]
    amx = np.repeat(am, WIN, axis=-1)  # [B,T,h,qw,256]
    mw16 = np.zeros((B, T, 128, 2, 2, 128), np.float32)
    for jbq in range(2):
        for rg in range(4):
            h = 4 * jbq + rg
            for kb in range(2):
                # rows 32rg+w (32-aligned lhsT base), free dims (jbq, kb)
                mw16[:, :, 32 * rg:32 * rg + 8, jbq, kb, :] = \
                    amx[:, :, h, :, kb * 128:(kb + 1) * 128]
    return mw16.astype(ml_dtypes.bfloat16)


def _make_e16r():
    e = np.zeros((128, S), ml_dtypes.bfloat16)
    q = np.arange(S) // WIN  # query window of column q
    for rg in range(4):
        for w in range(NW):
            e[32 * rg + w, q == w] = 1.0
    return e


def _host_prep(x, w_qkv, b_qkv, w_proj, b_proj):
    bf16 = ml_dtypes.bfloat16
    x4 = x.reshape(B, T, S, C)
    xt = np.ascontiguousarray(x4.transpose(0, 1, 3, 2)).astype(bf16)
    mw16 = _host_routing(x4, w_qkv, b_qkv)

    bfinal = b_qkv[2 * C:] @ w_proj + b_proj
    shared = {
        "wqk_bf": np.ascontiguousarray(w_qkv[:, :2 * C]).astype(bf16),
        "wv_bf": np.ascontiguousarray(w_qkv[:, 2 * C:]).astype(bf16),
        "wproj_bf": w_proj.astype(bf16),
        "e16r": _make_e16r(),
        "bqk_cols": np.ascontiguousarray(
            b_qkv[:2 * C].reshape(4, 128).T).astype(np.float32),
        "bfinal_row": bfinal.reshape(1, C).astype(bf16),
    }
    in_maps = []
    for core in range(NCORES):
        b0 = core * BPC
        m = dict(shared)
        m["xt"] = np.ascontiguousarray(xt[b0:b0 + BPC])
        m["mw16"] = np.ascontiguousarray(mw16[b0:b0 + BPC])
        in_maps.append(m)
    return in_maps


def kernel(x, w_qkv, b_qkv, w_proj, b_proj, **_unused_scalars):
    x = np.asarray(x, dtype=np.float32)
    w_qkv = np.asarray(w_qkv, dtype=np.float32)
    b_qkv = np.asarray(b_qkv, dtype=np.float32)
    w_proj = np.asarray(w_proj, dtype=np.float32)
    b_proj = np.asarray(b_proj, dtype=np.float32)

    has_bqk = bool(np.any(b_qkv[:2 * C]))
    bfinal = b_qkv[2 * C:] @ w_proj + b_proj
    has_bf = bool(np.any(bfinal))
    key = ("nc", has_bqk, has_bf)
    if key not in _CACHE:
        _CACHE[key] = _build_nc(has_bqk, has_bf)
        _CACHE["nc"] = _CACHE[key]
    nc = _CACHE[key]

    in_maps = _host_prep(x, w_qkv, b_qkv, w_proj, b_proj)
    res = run_bass_kernel_spmd(nc, in_maps, core_ids=list(range(NCORES)))

    out = np.empty((B, T, 2, 128, C), np.float32)
    for core in range(NCORES):
        out[core * BPC:(core + 1) * BPC] = res.results[core]["out"]
    # [B, T, sb, p, C] -> [B, T*S, C]
    return out.reshape(B, T * S, C)
